# revision 20
# baseline (speedup 1.0000x reference)
"""DenseSNN Trainium2 kernel: 4-layer LIF SNN, T=100, B=128, D=H=2048, C=100.

Strategy
--------
Layer-unrolled phases (layer-l spikes at step t depend only on layer-(l-1)
spikes at steps <= t):

    CUR_l = S_{l-1} @ W_l + b_l   (full-size GEMM over all T*B rows)
    S_l   = LIF-scan_T(CUR_l)     (elementwise in (B,H), sequential in T)

Data-parallel over batch: 16 samples/core on 8 cores, communication-free.

On-chip layout is "transposed activations": [feature -> 16 chunks x 128
partitions, (t,b) -> free axis]; the host pre-transposes x and re-assembles
the output, so the device never transposes.

Matmuls run in fp8(e4m3) with perf_mode=DoubleRow (two 128-row fp8 matmuls
fused per instruction, K=256 per MM). Weights are host-prescaled by 1024
(raw weights ~±0.022 sit below the e4m3 min-normal 2^-6), x by 16; the
PSUM->SBUF activation rescales and adds the bias. Spikes are 0/1 — exact
in fp8. Row slices are 4x400 (even): at N=400 the ~190ns stream time
roughly matches the 256-col DoubleRow LDWEIGHTS, so weight loads hide; a
3x512+64 split would leave a LDWEIGHTS-bound tail.

LIF runs on the vector engine in bf16, 3 ops/step/layer:
    m = beta*m + cur     (cur written per-step-contiguous by the activation)
    m = m - r[t-1]       (reset by subtraction; skipped at t=0)
    r[t] = (m > 1)       (-> small bf16 ring, 2 x 5-step batches)
Spike fp8 conversion happens OFF the DVE critical chain as 5-step batched
casts: gpsimd for layers 1-2 (large slack), scalar for layer 3 (short
slack; gpsimd per-op overhead is ~1us and its SBUF port contends with the
DVE, so it only gets the low-rate batched work).

The output layer (C=100) is interleaved with layer 3, shifted one slice
later so the layer-3 spike casts have a full slice of slack before the
output matmuls consume them.
"""

import os
import numpy as np
import ml_dtypes

import concourse.bass as bass
import concourse.mybir as mybir
import concourse.tile as tile
from concourse import bacc
from concourse.bass_utils import run_bass_kernel_spmd

# Problem constants (hardcoded per contract)
T, B, D, H, C = 100, 128, 2048, 2048, 100
NCORES = 8
BC = B // NCORES          # 16 samples per core
R = T * BC                # 1600 rows (t,b) per core
KC = D // 128             # 16 contraction chunks of 128
KK = KC // 2              # 8 DoubleRow chunk-pairs (K=256 per matmul)
HC = H // 128             # 16 output-feature chunks
BETA = 0.9
XSCALE = 16.0             # x pre-scale before fp8 cast
WSCALE = 1024.0           # weight pre-scale before fp8 cast
NR = 400                  # row-slice width (25 steps x BC)
NS = R // NR              # 4 slices
NT = NR // BC             # 25 steps per slice
CB = 5                    # spike-cast batch (steps per fp8 cast)
SLICES = [(r0, NR) for r0 in range(0, R, NR)]

_DEBUG_SPIKES = bool(os.environ.get("SNN_DEBUG_SPIKES"))
F32 = mybir.dt.float32
BF16 = mybir.dt.bfloat16
FP8 = mybir.dt.float8e4
ALU = mybir.AluOpType
ACTF = mybir.ActivationFunctionType
DR = mybir.MatmulPerfMode.DoubleRow


def _build_nc():
    nc = bacc.Bacc("TRN2", target_bir_lowering=False)

    xT_d = nc.dram_tensor("xT", [KK, 128, R, 2], FP8, kind="ExternalInput")
    w_d = [
        nc.dram_tensor("w1", [D, H], FP8, kind="ExternalInput"),
        nc.dram_tensor("w2", [H, H], FP8, kind="ExternalInput"),
        nc.dram_tensor("w3", [H, H], FP8, kind="ExternalInput"),
    ]
    wo_d = nc.dram_tensor("wo", [H, C], FP8, kind="ExternalInput")
    bias_d = nc.dram_tensor("biases", [128, 3 * HC], F32, kind="ExternalInput")
    bo_d = nc.dram_tensor("biaso", [C, 1], F32, kind="ExternalInput")
    out_d = nc.dram_tensor("out", [C, BC], F32, kind="ExternalOutput")

    with tile.TileContext(nc) as tc:
        with (
            tc.tile_pool(name="wpool", bufs=2) as wpool,       # 2x 32KB
            tc.tile_pool(name="spool", bufs=2) as spool,       # S1,S2 25.6KB ea
            tc.tile_pool(name="s3pool", bufs=2) as s3pool,     # 6.25KB ea
            tc.tile_pool(name="curpool", bufs=3) as curpool,   # 12.5KB ea
            tc.tile_pool(name="copool", bufs=2) as copool,     # 0.8KB ea
            tc.tile_pool(name="xpool", bufs=2) as xpool,       # 6.25KB ea
            tc.tile_pool(name="small", bufs=1) as small,
            tc.tile_pool(name="pspool", bufs=7, space="PSUM") as pspool,
        ):
            # Persistent big tensors
            S1 = spool.tile([128, KC * R], FP8, tag="S")
            S2 = spool.tile([128, KC * R], FP8, tag="S")
            w_sb = [None, None]  # rotating slots
            # wo padded to 128 cols/chunk: DoubleRow LDWEIGHTS needs the
            # pair-dim stride %16==0 (C=100 is not); pad cols feed unused
            # PSUM partitions 100..127.
            wo_sb = small.tile([128, KC * 128], FP8)

            # Small state
            mstate = small.tile([128, 3 * 256], BF16)
            # spike ring: per layer 2 batches x CB steps x 256, (c,b) order
            rring = small.tile([128, 3 * 2 * CB * 256], BF16)
            bias_sb = small.tile([128, 3 * HC], F32)
            ost = small.tile([128, 64], F32)   # output-layer state
            memo = ost[:C, 0:16]
            ssum = ost[:C, 16:32]
            roring = [ost[:C, 32:48], ost[:C, 48:64]]
            bo_sb = small.tile([C, 1], F32)

            def m_of(li):
                return mstate[:, li * 256:(li + 1) * 256]

            def r_of(li, t):
                off = (li * 2 * CB + ((t // CB) % 2) * CB + t % CB) * 256
                return rring[:, off:off + 256]

            nc.gpsimd.memset(mstate[:], 0.0)
            nc.gpsimd.memset(ost[:], 0.0)
            nc.gpsimd.memset(wo_sb[:], 0.0)

            # ---- Upfront DMAs. Descriptor-gen is ~650ns each; sync carries
            # only the latency-critical w1, gpsimd everything else.
            w_sb[0] = wpool.tile([128, KC * H], FP8, tag="W", name="w_a")
            for kc in range(KC):
                nc.sync.dma_start(
                    w_sb[0][:, kc * H:(kc + 1) * H],
                    w_d[0][kc * 128:(kc + 1) * 128, :],
                )
            nc.gpsimd.dma_start(bias_sb[:], bias_d[:])
            nc.gpsimd.dma_start(bo_sb[:], bo_d[:])
            w_sb[1] = wpool.tile([128, KC * H], FP8, tag="W", name="w_b")
            for kc in range(KC):
                nc.gpsimd.dma_start(
                    w_sb[1][:, kc * H:(kc + 1) * H],
                    w_d[1][kc * 128:(kc + 1) * 128, :],
                )
            for kc in range(KC):
                nc.gpsimd.dma_start(
                    wo_sb[:, kc * 128:kc * 128 + C],
                    wo_d[kc * 128:(kc + 1) * 128, :],
                )

            S1_3 = S1.rearrange("p (c r) -> p c r", c=KC)
            S2_3 = S2.rearrange("p (c r) -> p c r", c=KC)

            def matmul_slice(w, rhs3, nr, cur, li):
                """16 output chunks x 8 DoubleRow MMs; bias/rescale writes cur
                in per-step layout [p, t, (c b)] so LIF reads contiguously."""
                w3 = w.rearrange("p (c h) -> p c h", c=KC)
                cur3 = cur.rearrange("p (t x) -> p t x", t=NT)
                scale = 1.0 / (XSCALE * WSCALE) if li == 0 else 1.0 / WSCALE
                for hc in range(HC):
                    ps = pspool.tile([128, 512], F32, tag="ps", name="ps")
                    for kk in range(KK):
                        nc.tensor.matmul(
                            ps[:, :nr],
                            w3[:, 2 * kk:2 * kk + 2, hc * 128:(hc + 1) * 128],
                            rhs3(kk),
                            start=(kk == 0),
                            stop=(kk == KK - 1),
                            perf_mode=DR,
                        )
                    nc.scalar.activation(
                        cur3[:, :, hc * BC:(hc + 1) * BC],
                        ps[:, :nr].rearrange("p (t b) -> p t b", t=NT),
                        ACTF.Identity,
                        bias=bias_sb[:, li * HC + hc: li * HC + hc + 1],
                        scale=scale,
                    )

            def cast_batch(li, t0, S_3, s_t0):
                """fp8-convert CB steps of spikes from the bf16 ring into the
                spike tensor (matmul rhs layout [p, c, r])."""
                base = (li * 2 * CB + ((t0 // CB) % 2) * CB) * 256
                src = rring[:, base:base + CB * 256].rearrange(
                    "p (s c b) -> p s c b", s=CB, c=KC)
                w0 = (t0 - s_t0) * BC
                dst = S_3[:, :, w0:w0 + CB * BC].rearrange(
                    "p c (s b) -> p s c b", s=CB)
                # scalar, not gpsimd: gpsimd's per-op overhead is ~3x and its
                # SBUF port contends with the DVE's
                nc.scalar.copy(dst, src)

            def lif_slice(li, r0, cur, S_3, s_t0):
                m = m_of(li)
                for tl in range(NT):
                    t = r0 // BC + tl
                    nc.vector.scalar_tensor_tensor(
                        m, m, BETA, cur[:, tl * 256:(tl + 1) * 256],
                        ALU.mult, ALU.add
                    )
                    if t > 0:
                        nc.vector.tensor_tensor(m, m, r_of(li, t - 1), ALU.subtract)
                    nc.vector.tensor_scalar(r_of(li, t), m, 1.0, None, ALU.is_gt)
                    if t % CB == CB - 1:
                        cast_batch(li, t - CB + 1, S_3, s_t0)

            def out_slice(s3_3, r0, nr):
                """Output layer for one slice: matmul + LIF + spike-count."""
                pso = pspool.tile([128, 512], F32, tag="ps", name="pso")
                for kk in range(KK):
                    nc.tensor.matmul(
                        pso[:, :nr],
                        wo_sb.rearrange("p (c h) -> p c h", c=KC)[
                            :, 2 * kk:2 * kk + 2, :],
                        s3_3[:, 2 * kk:2 * kk + 2, :nr],
                        start=(kk == 0),
                        stop=(kk == KK - 1),
                        perf_mode=DR,
                    )
                curo = copool.tile([128, NR], BF16, tag="curo", name="curo")
                curo_f = curo[:C, :nr]
                nc.scalar.activation(
                    curo_f, pso[:C, :nr], ACTF.Identity,
                    bias=bo_sb, scale=1.0 / WSCALE,
                )
                for tl in range(NT):
                    t = r0 // BC + tl
                    cur_t = curo_f[:, tl * BC:(tl + 1) * BC]
                    nc.vector.scalar_tensor_tensor(
                        memo, memo, BETA, cur_t, ALU.mult, ALU.add
                    )
                    if t > 0:
                        nc.vector.tensor_tensor(
                            memo, memo, roring[(t - 1) % 2], ALU.subtract
                        )
                    nc.vector.tensor_scalar(
                        roring[t % 2], memo, 1.0, None, ALU.is_gt
                    )
                    nc.vector.tensor_tensor(ssum, ssum, roring[t % 2], ALU.add)

            # ---- Layer 1: rhs streamed from HBM (x^T, host-pretransposed).
            # x DMAs prefetch one slice ahead on the gpsimd queue.
            xin_t = [None] * NS

            def xin_fetch(j):
                xin_t[j] = xpool.tile([128, KC * NR], FP8, tag="x", name="xin")
                for kk in range(KK):
                    nc.gpsimd.dma_start(
                        xin_t[j][:, kk * 2 * NR:(kk + 1) * 2 * NR],
                        xT_d[kk][:, j * NR:(j + 1) * NR, :],
                    )

            xin_fetch(0)
            for j, (r0, nr) in enumerate(SLICES):
                # x is host-interleaved so each DoubleRow contraction pair is
                # adjacent in memory: layout [p, kk, r, i], rhs AP [p, i, r]
                xin3 = xin_t[j].rearrange("p (k r i) -> p k i r", k=KK, i=2)
                cur = curpool.tile([128, NT * 256], BF16, tag="cur", name="cur")
                matmul_slice(w_sb[0], lambda kk: xin3[:, kk],
                             nr, cur, 0)
                if j + 1 < NS:
                    xin_fetch(j + 1)
                lif_slice(0, r0, cur, S1_3, 0)

            # prefetch w3 into slot 0 (WAR on layer-1 matmuls, auto-tracked)
            w_sb[0] = wpool.tile([128, KC * H], FP8, tag="W", name="w_c")
            for kc in range(KC):
                nc.gpsimd.dma_start(
                    w_sb[0][:, kc * H:(kc + 1) * H],
                    w_d[2][kc * 128:(kc + 1) * 128, :],
                )

            # ---- Layer 2
            for r0, nr in SLICES:
                cur = curpool.tile([128, NT * 256], BF16, tag="cur", name="cur")
                matmul_slice(w_sb[1],
                             lambda kk: S1_3[:, 2 * kk:2 * kk + 2, r0:r0 + nr],
                             nr, cur, 1)
                lif_slice(1, r0, cur, S2_3, 0)

            # ---- Layer 3 + output layer, interleaved one slice behind
            prev = None
            for r0, nr in SLICES:
                cur = curpool.tile([128, NT * 256], BF16, tag="cur", name="cur")
                matmul_slice(w_sb[0],
                             lambda kk: S2_3[:, 2 * kk:2 * kk + 2, r0:r0 + nr],
                             nr, cur, 2)
                if prev is not None:
                    out_slice(*prev)
                s3 = s3pool.tile([128, KC * NR], FP8, tag="S3", name="s3")
                s3_3 = s3.rearrange("p (c r) -> p c r", c=KC)
                lif_slice(2, r0, cur, s3_3, r0 // BC)
                prev = (s3_3, r0, nr)
            out_slice(*prev)

            nc.sync.dma_start(out_d[:], ssum)

            if _DEBUG_SPIKES:
                for nm, S in (("s1_dbg", S1), ("s2_dbg", S2)):
                    sd = nc.dram_tensor(nm, [128, KC * R], FP8,
                                        kind="ExternalOutput")
                    nc.sync.dma_start(sd[:], S[:])

    nc.compile()
    return nc


_NC_CACHE = None


def _get_nc():
    global _NC_CACHE
    if _NC_CACHE is None:
        _NC_CACHE = _build_nc()
    return _NC_CACHE


def _fp8(a, scale):
    a = np.asarray(a, np.float32) * scale
    return np.ascontiguousarray(
        np.clip(a, -240.0, 240.0).astype(ml_dtypes.float8_e4m3)
    )


def make_in_maps(x_seq, W1, b1, W2, b2, W3, b3, Wo, bo):
    w1 = _fp8(W1, WSCALE)
    w2 = _fp8(W2, WSCALE)
    w3 = _fp8(W3, WSCALE)
    wo = _fp8(Wo, WSCALE)
    biases = np.concatenate(
        [np.asarray(b, np.float32).reshape(HC, 128).T for b in (b1, b2, b3)],
        axis=1,
    )
    biases = np.ascontiguousarray(biases)                 # [128, 48]
    bo_a = np.ascontiguousarray(np.asarray(bo, np.float32).reshape(C, 1))
    in_maps = []
    for c in range(NCORES):
        xs = np.asarray(x_seq[:, c * BC:(c + 1) * BC, :], np.float32)
        xT = xs.transpose(2, 0, 1).reshape(KC, 128, R)    # [kc, p, r]
        # pair-interleave contraction pairs: [kk, p, r, i], i = kc % 2
        xTi = xT.reshape(KK, 2, 128, R).transpose(0, 2, 3, 1)
        in_maps.append({
            "xT": _fp8(np.ascontiguousarray(xTi), XSCALE),
            "w1": w1, "w2": w2, "w3": w3, "wo": wo,
            "biases": biases, "biaso": bo_a,
        })
    return in_maps


def kernel(x_seq, W1, b1, W2, b2, W3, b3, Wo, bo):
    nc = _get_nc()
    in_maps = make_in_maps(x_seq, W1, b1, W2, b2, W3, b3, Wo, bo)
    res = run_bass_kernel_spmd(nc, in_maps, core_ids=list(range(NCORES)))
    outs = [res.results[c]["out"] for c in range(NCORES)]   # each [C, BC]
    return np.concatenate([o.T for o in outs], axis=0).astype(np.float32)
